# revision 1
# baseline (speedup 1.0000x reference)
"""AttentionPooling v4: ship x once, transpose on-chip, host-side scatter.

z[b] = sum_i softmax_within_segment(alpha)_i * x_i, alpha = tanh(x@W1.T)@W2.T.

vs the v1 baseline (shipped x in BOTH orientations = 64MB/core, DMA-bound,
sim 221.9us / HW 245812ns): ship only x_nat (bf16, 32MB/core) and build the
transposed copy on-chip:

- per 4-tile quad: the PE transposes x_nat 128x128 blocks into PSUM (bf16),
  then DVE (1 in 8 quads: ACT, for balance) copies them to SBUF; mm1
  (y^T = W1 @ x^T, K=256 in 2 chunks) consumes that copy.
- tanh (ACT) -> th bf16; mm2 alpha = th^T @ W2 per 128-row tile (PE, out
  [128,1]); exp batched per group (1 ACT op); weighted one-hot E built per
  group (2 DVE ops); the one-hot width EW is chosen per-run from the data
  (max segments any group spans +2, rounded to a multiple of 4 -- 20 here;
  narrower EW shrinks both the pool matmul free size and the E-build).
- pooling FLIPPED: gp^T[d_chunk, ew] += x_chunk^T @ E with x_nat as the
  stationary operand (out free size 32, 4x cheaper than the unflipped form).
  The per-group gp^T (32KB) is evacuated and DMA'd straight to DRAM; the
  HOST does the exact 0/1 f32 scatter into per-segment sums. Doing the
  scatter on-device costs a PE->ACT->PE->ACT chain that stalls the in-order
  PE ~1.9us every group (and 2 PSUM banks).
- groups: small leading sizes (8,8,16,32) to shorten the pipeline-fill
  dependency ladder, GT=32 steady state, small trailing sizes; x_nat DMA'd
  in 6-tile pieces so compute starts as data lands.
- host: z = scatter(gp^T dumps) / denominators(e dump), both exact.

Engine busy per core (CoreSim, span 121.6us): PE 112.4 at 90% occupancy
(transposes 53 + mm1 53 + pool 8), SP/DMA 106.6, DVE 93.1 (x^T evacs +
E-build), ACT 98.0 (tanh + exp + evac share).
"""

import numpy as np
import ml_dtypes

import concourse.bacc as bacc
import concourse.mybir as mybir
import concourse.tile as tile
from concourse.bass_utils import run_bass_kernel_spmd

bf16 = ml_dtypes.bfloat16
F32 = mybir.dt.float32
BF16 = mybir.dt.bfloat16
AF = mybir.ActivationFunctionType
ALU = mybir.AluOpType

NCORES = 8
D = 256
H = 128
SEGS_PER_CORE = 128
GT = 32          # tiles per group (a group's segments must fit mod-EW)
EW_CAP = 32      # one-hot width: chosen per-run from the data (max segments
                 # any group spans, plus margin), capped here
QUAD = 4         # tiles per mm1/psum batch
DMA_PIECE = 6    # tiles per xn DMA transfer (finer pieces pipeline better)

# every XBAR_EVERY-th quad is transposed by the DMA xbar instead of the PE
# (disabled: per-block issue overhead ~650ns dwarfs the 420ns PE saving, and
# the issuing engine's SEQ blocks on the transpose's data waits)
XBAR_EVERY = 10 ** 9
XBAR_PHASE = 3

_kernel_cache = {}


def _group_plan(nt):
    # small leading groups so exp/E-build/pool start early (shorter
    # dependency ladder during pipeline fill), then full GT groups, then
    # small trailing groups (shorter drain chain after the last tile)
    sizes = []
    left = nt
    for s in (8, 8, 16, 32):
        if left >= s + 16:
            sizes.append(s)
            left -= s
    while left >= GT + 16:
        sizes.append(GT)
        left -= GT
    for s in (32, 16, 8, 4):
        while left >= s and (s == 4 or left >= s + 4 or left == s):
            sizes.append(s)
            left -= s
    assert sum(sizes) == nt and left == 0, (sizes, nt)
    return sizes


def _build_kernel(nt, EW):
    assert nt % QUAD == 0
    sizes = _group_plan(nt)
    ngroups = len(sizes)

    nc = bacc.Bacc("TRN2", target_bir_lowering=False, debug=False)

    xn_d = nc.dram_tensor("x_nat", [128, nt, D], BF16, kind="ExternalInput").ap()
    ci_d = nc.dram_tensor("colidx", [128, nt], BF16, kind="ExternalInput").ap()
    w1t_d = nc.dram_tensor("W1T", [128, 2, H], BF16, kind="ExternalInput").ap()
    w2_d = nc.dram_tensor("W2c", [H, 1], BF16, kind="ExternalInput").ap()
    iota_d = nc.dram_tensor("iota", [128, EW], BF16, kind="ExternalInput").ap()
    ident_d = nc.dram_tensor("ident", [128, 128], BF16, kind="ExternalInput").ap()
    gpt_d = nc.dram_tensor("gptout", [len(sizes), 128, 2, EW], F32,
                           kind="ExternalOutput").ap()
    e_out_d = nc.dram_tensor("e_out", [128, nt], BF16, kind="ExternalOutput").ap()

    with tile.TileContext(nc) as tc:
        with (
            tc.tile_pool(name="const", bufs=1) as constp,
            tc.tile_pool(name="xn", bufs=4) as xnp,
            tc.tile_pool(name="xt", bufs=6) as xtp,
            tc.tile_pool(name="th", bufs=4) as thp,
            tc.tile_pool(name="e4", bufs=2) as e4p,
            tc.tile_pool(name="gps", bufs=2) as gpsp,
            tc.tile_pool(name="out", bufs=1) as outp,
            tc.tile_pool(name="psum_y", bufs=2, space="PSUM") as psumy,
            tc.tile_pool(name="psum_xt", bufs=3, space="PSUM") as psumxt,
            tc.tile_pool(name="psum_al", bufs=1, space="PSUM") as psumal,
            tc.tile_pool(name="psum_gt", bufs=2, space="PSUM") as psumgt,
        ):
            # first group's x is loaded per-quad ahead of the other consts so
            # the PE can start transposing ~2us in instead of ~14us
            ident_sb = constp.tile([128, 128], BF16)
            nc.sync.dma_start(ident_sb[:], ident_d[:])
            g0size = sizes[0]
            xn0 = xnp.tile([128, g0size, D], BF16, tag="xn",
                           padded_shape=[128, GT, D])
            nc.sync.dma_start(xn0[:, 0:QUAD, :], xn_d[:, 0:QUAD, :])
            w1t_sb = constp.tile([128, 2, H], BF16)
            nc.sync.dma_start(w1t_sb[:], w1t_d[:])
            w2_sb = constp.tile([H, 1], BF16)
            nc.sync.dma_start(w2_sb[:], w2_d[:])
            for q0 in range(QUAD, g0size, QUAD):
                qn = min(QUAD, g0size - q0)
                nc.sync.dma_start(xn0[:, q0:q0 + qn, :],
                                  xn_d[:, q0:q0 + qn, :])
            iota_sb = constp.tile([128, EW], BF16)
            nc.sync.dma_start(iota_sb[:], iota_d[:])
            ci_sb = constp.tile([128, nt], BF16)
            nc.sync.dma_start(ci_sb[:], ci_d[:])
            e_buf = constp.tile([128, nt], BF16)

            # deferred pooling closures, run one group behind compute.
            # The per-group pool gp^T goes straight to DRAM; the host does the
            # (exact, f32, 0/1) scatter into per-segment sums -- this keeps
            # the in-order PE from stalling through a PE->ACT->PE->ACT chain
            # every group.
            pending_pool = []

            def emit_pool(g, gsize, gstart, xn, e4g):
                gpt_ps = psumgt.tile([128, 2, EW], F32, tag="gpt")
                for c in range(2):
                    for tg in range(gsize):
                        nc.tensor.matmul(
                            gpt_ps[:, c, :],
                            xn[:, tg, c * 128:(c + 1) * 128],
                            e4g[:, tg, :],
                            start=(tg == 0), stop=(tg == gsize - 1))
                gpt_sb = gpsp.tile([128, 2, EW], F32, tag="gpt_sb")
                nc.scalar.activation(gpt_sb[:], gpt_ps[:], AF.Copy)
                nc.sync.dma_start(gpt_d[g, :, :, :], gpt_sb[:])

            gstart = 0
            evac_i = 0
            for g, gsize in enumerate(sizes):
                if g == 0:
                    xn = xn0
                else:
                    xn = xnp.tile([128, gsize, D], BF16, tag="xn",
                                  padded_shape=[128, GT, D])
                    for p0 in range(0, gsize, DMA_PIECE):
                        pn = min(DMA_PIECE, gsize - p0)
                        nc.sync.dma_start(
                            xn[:, p0:p0 + pn, :],
                            xn_d[:, gstart + p0:gstart + p0 + pn, :])

                al_ps = psumal.tile([128, gsize], F32, tag="al",
                                    padded_shape=[128, GT])

                for q0 in range(0, gsize, QUAD):
                    qn = min(QUAD, gsize - q0)
                    xt_sb = xtp.tile([128, 2, qn * 128], BF16, tag="xt_sb",
                                     padded_shape=[128, 2, QUAD * 128])
                    if False:
                        # xbar-DMA transpose straight to SBUF (no PSUM, no
                        # evac); issued on the ACT/DVE queues to keep the SP
                        # sequencer free
                        eng = nc.scalar if (evac_i // XBAR_EVERY) % 2 == 0 \
                            else nc.sync
                        for j in range(qn):
                            t = q0 + j
                            for c in range(2):
                                eng.dma_start_transpose(
                                    xt_sb[:, c, j * 128:(j + 1) * 128],
                                    xn[:, t, c * 128:(c + 1) * 128])
                    else:
                        # PE transpose into PSUM (bf16), evac via DVE
                        xt_ps = psumxt.tile([128, 2, qn * 128], BF16,
                                            tag="xt_ps",
                                            padded_shape=[128, 2, QUAD * 128])
                        for j in range(qn):
                            t = q0 + j
                            for c in range(2):
                                nc.tensor.transpose(
                                    xt_ps[:, c, j * 128:(j + 1) * 128],
                                    xn[:, t, c * 128:(c + 1) * 128],
                                    ident_sb[:])
                        if evac_i % 8 == 5:
                            nc.scalar.activation(xt_sb[:], xt_ps[:], AF.Copy)
                        else:
                            nc.vector.tensor_copy(xt_sb[:], xt_ps[:])
                    evac_i += 1

                    # mm1: y^T = W1c @ x^T  (accumulate over 2 chunks)
                    y_ps = psumy.tile([128, qn * 128], F32, tag="y",
                                      padded_shape=[128, QUAD * 128])
                    for c in range(2):
                        nc.tensor.matmul(y_ps[:], w1t_sb[:, c, :],
                                         xt_sb[:, c, :],
                                         start=(c == 0), stop=(c == 1))
                    th = thp.tile([128, qn * 128], BF16, tag="th",
                                  padded_shape=[128, QUAD * 128])
                    nc.scalar.activation(th[:], y_ps[:], AF.Tanh)
                    for j in range(qn):
                        nc.tensor.matmul(al_ps[:, q0 + j:q0 + j + 1],
                                         th[:, j * 128:(j + 1) * 128],
                                         w2_sb[:], start=True, stop=True)

                nc.scalar.activation(e_buf[:, gstart:gstart + gsize], al_ps[:],
                                     AF.Exp)

                # one-hot E (weighted by e) for the whole group: 2 DVE ops
                s4 = e4p.tile([128, gsize, EW], BF16, tag="s4",
                              padded_shape=[128, GT, EW])
                nc.vector.tensor_tensor(
                    s4[:],
                    ci_sb[:, gstart:gstart + gsize].broadcast_to(
                        [128, gsize, EW]),
                    iota_sb[:, None, :].broadcast_to([128, gsize, EW]),
                    ALU.is_equal)
                e4g = e4p.tile([128, gsize, EW], BF16, tag="e4g",
                               padded_shape=[128, GT, EW])
                nc.vector.tensor_mul(
                    e4g[:], s4[:],
                    e_buf[:, gstart:gstart + gsize].broadcast_to(
                        [128, gsize, EW]))

                # run the PREVIOUS group's pooling now (keeps PE from
                # stalling on this group's E4)
                for fn in pending_pool:
                    fn()
                pending_pool = [
                    (lambda g=g, gsize=gsize, gstart=gstart, xn=xn, e4g=e4g:
                     emit_pool(g, gsize, gstart, xn, e4g))]
                gstart += gsize

            for fn in pending_pool:
                fn()

            nc.sync.dma_start(e_out_d[:], e_buf[:])

    nc.compile()
    return nc


def _prep_core(x, batch, r0, r1, seg0, nt, EW):
    rows = r1 - r0
    pad_rows = nt * 128

    xb = np.zeros((pad_rows, D), dtype=bf16)
    xb[:rows] = x[r0:r1].astype(bf16)
    # (128, nt, D): partition p holds row t*128 + p
    x_nat = np.ascontiguousarray(xb.reshape(nt, 128, D).transpose(1, 0, 2))

    seg_local = np.full(pad_rows, -1, dtype=np.int64)
    seg_local[:rows] = batch[r0:r1] - seg0
    ci = np.where(seg_local < 0, -1.0, seg_local % EW).astype(np.float32)
    colidx = np.ascontiguousarray(ci.reshape(nt, 128).T).astype(bf16)

    sizes = _group_plan(nt)
    gsegs = []  # per group: local segment ids present
    gstart = 0
    for g, gsize in enumerate(sizes):
        segs = np.unique(seg_local[gstart * 128:(gstart + gsize) * 128])
        segs = segs[segs >= 0]
        assert segs.size <= EW, f"group {g} spans {segs.size} segments > EW"
        gsegs.append(segs)
        gstart += gsize

    return {"x_nat": x_nat, "colidx": colidx}, gsegs


def _shared_inputs(W1, W2, EW):
    w1t = np.ascontiguousarray(
        W1.T.astype(bf16).reshape(2, H, H).transpose(1, 0, 2))
    w2c = np.ascontiguousarray(W2.reshape(H, 1).astype(bf16))
    iota = np.broadcast_to(
        np.arange(EW, dtype=np.float32), (128, EW)).astype(bf16)
    ident = np.eye(128, dtype=bf16)
    return {"W1T": w1t, "W2c": w2c, "iota": iota, "ident": ident}


def _seg_starts(x, batch):
    s = np.searchsorted(batch, np.arange(0, NCORES * SEGS_PER_CORE + 1,
                                         SEGS_PER_CORE))
    s[0], s[-1] = 0, x.shape[0]
    return s


def build_in_maps(x, batch, nt, EW=None):
    if EW is None:
        EW = pick_ew(x, batch, nt)
    s = _seg_starts(x, batch)
    maps, gsegs = [], []
    for c in range(NCORES):
        m, gs = _prep_core(x, batch, int(s[c]), int(s[c + 1]),
                           c * SEGS_PER_CORE, nt, EW)
        maps.append(m)
        gsegs.append(gs)
    return maps, gsegs


def pick_ew(x, batch, nt):
    """One-hot width: max segments any group spans on any core, +2 margin,
    rounded up to a multiple of 4 (capped at EW_CAP)."""
    s = _seg_starts(x, batch)
    sizes = _group_plan(nt)
    worst = 1
    for c in range(NCORES):
        r0, r1 = int(s[c]), int(s[c + 1])
        seg_local = np.full(nt * 128, -1, dtype=np.int64)
        seg_local[:r1 - r0] = np.asarray(batch[r0:r1]) - SEGS_PER_CORE * c
        gstart = 0
        for gsize in sizes:
            segs = np.unique(seg_local[gstart * 128:(gstart + gsize) * 128])
            worst = max(worst, (segs >= 0).sum())
            gstart += gsize
    return min(EW_CAP, -(-(int(worst) + 2) // 4) * 4)


def pick_nt(x, batch):
    s = _seg_starts(x, batch)
    nt = int(max(-(-(int(s[c + 1] - s[c])) // 128) for c in range(NCORES)))
    return -(-nt // QUAD) * QUAD


def kernel(x, batch, W1, W2, B):
    x = np.asarray(x)
    batch = np.asarray(batch)
    W1 = np.asarray(W1)
    W2 = np.asarray(W2)
    B = int(B)
    assert B == NCORES * SEGS_PER_CORE

    nt = pick_nt(x, batch)
    ew = pick_ew(x, batch, nt)
    key = (nt, ew)
    if key not in _kernel_cache:
        _kernel_cache.clear()
        _kernel_cache[key] = _build_kernel(nt, ew)
    nc = _kernel_cache[key]

    shared = _shared_inputs(W1, W2, ew)
    in_maps, gsegs = build_in_maps(x, batch, nt, ew)
    for m in in_maps:
        m.update(shared)

    res = run_bass_kernel_spmd(nc, in_maps, core_ids=list(range(NCORES)))

    seg_starts = _seg_starts(x, batch)
    z = np.empty((B, D), dtype=np.float32)
    for c in range(NCORES):
        # host-side scatter: gptout[g, d_lo, chunk, ew] -> num[seg, d]
        gpt = res.results[c]["gptout"].astype(np.float64)
        num = np.zeros((SEGS_PER_CORE, D))
        for g, segs in enumerate(gsegs[c]):
            sel = gpt[g][:, :, segs % ew]          # (128, 2, nJ)
            num[segs] += sel.transpose(2, 1, 0).reshape(len(segs), D)
        num = num.astype(np.float32)
        e = res.results[c]["e_out"].T.reshape(-1)
        r0, r1 = int(seg_starts[c]), int(seg_starts[c + 1])
        seg_local = (batch[r0:r1] - c * SEGS_PER_CORE).astype(np.int64)
        e_rows = e[:r1 - r0].astype(np.float64)
        den = np.bincount(seg_local, weights=e_rows, minlength=SEGS_PER_CORE)
        den = np.where(den == 0.0, 1.0, den).astype(np.float32)
        z[c * SEGS_PER_CORE:(c + 1) * SEGS_PER_CORE] = num / den[:, None]
    return z



# revision 5
# speedup vs baseline: 1.1800x; 1.1800x over previous
"""AttentionPooling v5: fp8(e3m4) x, host-side partial pre-transpose.

z[b] = sum_i softmax_within_segment(alpha)_i * x_i, alpha = tanh(x@W1.T)@W2.T.

vs v4 (bf16 x_nat shipped once, ALL of x^T built on-chip by PE transposes:
PE busy 112.4us = transposes 53 + mm1 53 + pool 8, sim 121.7us):

- x ships as fp8 e3m4 (1B/elem). Accuracy: e3m4 keeps 4 mantissa bits;
  numpy-simulating the exact pipeline gives rel-err 1.57e-2 < 2e-2 gate
  (e4m3 would be 3.8e-2 -- fails). W1/W2 stay bf16 (W1 values ~N(0,1/256)
  are subnormal in e3m4: quantizing W1 to fp8 blows the error to 7.8e-2).
- x_nat (pooling stationary) is 16MB/core; 2 of every 3 quads ALSO ship a
  host-built x^T copy (fp8, +10.7MB); the PE transposes only the remaining
  third on-chip. This balances DMA (~78us) against PE (~80us = 17.5
  transposes + 52.5 mm1 + 9 pool) instead of v4's PE-bound 114us.
- the one-hot S4 = (colidx == iota) ships precomputed (fp8 0/1, 1.26MB)
  so the E-build is one DVE multiply per group instead of two ops.
- tanh runs once per quad-PAIR ([128,2,512] f32 PSUM -> [128,1024] bf16),
  halving ACT per-op overhead (ACT ~68us: tanh 64 + exp 3.6).
- evacs for the transposed third go to DVE (fp8 copies get no 2x mode:
  ~1.2us/quad but only 41 quads); gpt evac moved ACT->DVE. DVE ~64us.
- pooling/host-scatter/denominators unchanged from v4 (gp^T per group ->
  DRAM, host does the exact 0/1 f32 scatter + bincount denominators).

Predicted engine busy per core: DMA ~78, PE ~80, ACT ~68, DVE ~64.
"""

import numpy as np
import ml_dtypes

import concourse.bacc as bacc
import concourse.mybir as mybir
import concourse.tile as tile
from concourse.bass_utils import run_bass_kernel_spmd

bf16 = ml_dtypes.bfloat16
f8e3 = ml_dtypes.float8_e3m4
F32 = mybir.dt.float32
BF16 = mybir.dt.bfloat16
FP8E3 = mybir.dt.float8e3
AF = mybir.ActivationFunctionType
ALU = mybir.AluOpType

NCORES = 8
D = 256
H = 128
SEGS_PER_CORE = 128
GT = 32          # tiles per group (a group's segments must fit mod-EW)
EW_CAP = 32      # one-hot width cap
QUAD = 4         # tiles per mm1 batch
DMA_PIECE = 12   # tiles per x_nat DMA transfer
TRANSP_EVERY = 3  # 1 of every TRANSP_EVERY quads is transposed on-chip
TRANSP_PHASE = 2  # ... the one with global_quad % TRANSP_EVERY == this

_kernel_cache = {}


def _group_plan(nt):
    # small leading groups so exp/E-build/pool start early, then full GT
    # groups, then small trailing groups (shorter drain chain)
    sizes = []
    left = nt
    for s in (8, 8, 16, 32):
        if left >= s + 16:
            sizes.append(s)
            left -= s
    while left >= GT + 16:
        sizes.append(GT)
        left -= GT
    for s in (32, 16, 8, 4):
        while left >= s and (s == 4 or left >= s + 4 or left == s):
            sizes.append(s)
            left -= s
    assert sum(sizes) == nt and left == 0, (sizes, nt)
    return sizes


def _is_transposed_quad(qg):
    return qg % TRANSP_EVERY == TRANSP_PHASE


def _shipped_quads(nt):
    """Global quad indices whose x^T ships from the host, in order."""
    return [q for q in range(nt // QUAD) if not _is_transposed_quad(q)]


def _build_kernel(nt, EW):
    assert nt % QUAD == 0
    sizes = _group_plan(nt)
    nq_total = nt // QUAD
    shipped = _shipped_quads(nt)
    ship_off = {q: i for i, q in enumerate(shipped)}  # quad -> slab index

    nc = bacc.Bacc("TRN2", target_bir_lowering=False, debug=False)

    xn_d = nc.dram_tensor("x_nat", [128, nt, D], FP8E3, kind="ExternalInput").ap()
    xt_d = nc.dram_tensor("xT", [128, 2, len(shipped) * 512], FP8E3,
                          kind="ExternalInput").ap()
    s4_d = nc.dram_tensor("s4", [128, nt, EW], FP8E3, kind="ExternalInput").ap()
    w1t_d = nc.dram_tensor("W1T", [128, 2, H], BF16, kind="ExternalInput").ap()
    w2_d = nc.dram_tensor("W2c", [H, 1], BF16, kind="ExternalInput").ap()
    ident_d = nc.dram_tensor("ident", [128, 128], FP8E3, kind="ExternalInput").ap()
    gpt_d = nc.dram_tensor("gptout", [len(sizes), 128, 2, EW], F32,
                           kind="ExternalOutput").ap()
    e_out_d = nc.dram_tensor("e_out", [128, nt], BF16, kind="ExternalOutput").ap()

    with tile.TileContext(nc) as tc:
        with (
            tc.tile_pool(name="const", bufs=1) as constp,
            tc.tile_pool(name="xn", bufs=4) as xnp,
            tc.tile_pool(name="xts", bufs=3) as xtsp,    # shipped x^T slabs
            tc.tile_pool(name="xt", bufs=4) as xtp,      # transposed-quad x^T
            tc.tile_pool(name="th", bufs=4) as thp,
            tc.tile_pool(name="e4", bufs=2) as e4p,
            tc.tile_pool(name="gps", bufs=2) as gpsp,
            tc.tile_pool(name="psum_y", bufs=2, space="PSUM") as psumy,
            tc.tile_pool(name="psum_xt", bufs=2, space="PSUM") as psumxt,
            tc.tile_pool(name="psum_al", bufs=1, space="PSUM") as psumal,
            tc.tile_pool(name="psum_gt", bufs=1, space="PSUM") as psumgt,
        ):
            # first group's x loads ahead of the bulkier consts so compute
            # starts early
            ident_sb = constp.tile([128, 128], FP8E3)
            nc.sync.dma_start(ident_sb[:], ident_d[:])
            g0size = sizes[0]
            xn0 = xnp.tile([128, g0size, D], FP8E3, tag="xn",
                           padded_shape=[128, GT, D])
            nc.sync.dma_start(xn0[:, 0:QUAD, :], xn_d[:, 0:QUAD, :])
            w1t_sb = constp.tile([128, 2, H], BF16)
            nc.sync.dma_start(w1t_sb[:], w1t_d[:])
            w2_sb = constp.tile([H, 1], BF16)
            nc.sync.dma_start(w2_sb[:], w2_d[:])
            for q0 in range(QUAD, g0size, QUAD):
                qn = min(QUAD, g0size - q0)
                nc.sync.dma_start(xn0[:, q0:q0 + qn, :],
                                  xn_d[:, q0:q0 + qn, :])
            s4_sb = constp.tile([128, nt, EW], FP8E3)
            nc.sync.dma_start(s4_sb[:], s4_d[:])
            e_buf = constp.tile([128, nt], BF16)

            # deferred pooling closures, run one group behind compute
            pending_pool = []

            def emit_pool(g, gsize, gstart, xn, e4g):
                gpt_ps = psumgt.tile([128, 2, EW], F32, tag="gpt")
                for c in range(2):
                    for tg in range(gsize):
                        nc.tensor.matmul(
                            gpt_ps[:, c, :],
                            xn[:, tg, c * 128:(c + 1) * 128],
                            e4g[:, tg, :],
                            start=(tg == 0), stop=(tg == gsize - 1))
                gpt_sb = gpsp.tile([128, 2, EW], F32, tag="gpt_sb")
                nc.vector.tensor_copy(gpt_sb[:], gpt_ps[:])
                nc.sync.dma_start(gpt_d[g, :, :, :], gpt_sb[:])

            gstart = 0
            for g, gsize in enumerate(sizes):
                if g == 0:
                    xn = xn0
                else:
                    xn = xnp.tile([128, gsize, D], FP8E3, tag="xn",
                                  padded_shape=[128, GT, D])
                    for p0 in range(0, gsize, DMA_PIECE):
                        pn = min(DMA_PIECE, gsize - p0)
                        nc.sync.dma_start(
                            xn[:, p0:p0 + pn, :],
                            xn_d[:, gstart + p0:gstart + p0 + pn, :])

                # shipped x^T slab for this group (if any)
                gq0 = gstart // QUAD
                nquads = gsize // QUAD
                gship = [q for q in range(gq0, gq0 + nquads)
                         if not _is_transposed_quad(q)]
                if gship:
                    so = ship_off[gship[0]]
                    assert [ship_off[q] for q in gship] == \
                        list(range(so, so + len(gship)))
                    xts = xtsp.tile([128, 2, len(gship) * 512], FP8E3,
                                    tag="xts", padded_shape=[128, 2, 8 * 512])
                    nc.sync.dma_start(
                        xts[:], xt_d[:, :, so * 512:(so + len(gship)) * 512])

                al_ps = psumal.tile([128, gsize], F32, tag="al",
                                    padded_shape=[128, GT])

                # process quads in pairs sharing one y_ps / one tanh
                for p0 in range(0, nquads, 2):
                    pq = min(2, nquads - p0)
                    y_ps = psumy.tile([128, pq, 512], F32, tag="y",
                                      padded_shape=[128, 2, 512])
                    for pi in range(pq):
                        qg = gq0 + p0 + pi
                        q0 = (p0 + pi) * QUAD
                        if _is_transposed_quad(qg):
                            # fp8 transpose mode writes outputs at element
                            # step 2: interleave a dummy last dim
                            xt_ps = psumxt.tile(
                                [128, 2, 512, 2], FP8E3, tag="xt_ps")
                            for j in range(QUAD):
                                for c in range(2):
                                    nc.tensor.transpose(
                                        xt_ps[:, c, j * 128:(j + 1) * 128, 0],
                                        xn[:, q0 + j, c * 128:(c + 1) * 128],
                                        ident_sb[:])
                            xt_q = xtp.tile([128, 2, 512], FP8E3, tag="xt_sb")
                            nc.vector.tensor_copy(xt_q[:], xt_ps[:, :, :, 0])
                        else:
                            si = ship_off[qg] - ship_off[gship[0]]
                            xt_q = xts[:, :, si * 512:(si + 1) * 512]

                        # mm1: y^T = W1c @ x^T (accumulate over 2 K-chunks)
                        for c in range(2):
                            nc.tensor.matmul(y_ps[:, pi, :], w1t_sb[:, c, :],
                                             xt_q[:, c, :],
                                             start=(c == 0), stop=(c == 1))
                    th = thp.tile([128, pq, 512], BF16, tag="th",
                                  padded_shape=[128, 2, 512])
                    nc.scalar.activation(th[:], y_ps[:], AF.Tanh)
                    for pi in range(pq):
                        for j in range(QUAD):
                            t = (p0 + pi) * QUAD + j
                            nc.tensor.matmul(al_ps[:, t:t + 1],
                                             th[:, pi, j * 128:(j + 1) * 128],
                                             w2_sb[:], start=True, stop=True)

                nc.scalar.activation(e_buf[:, gstart:gstart + gsize], al_ps[:],
                                     AF.Exp)

                # weighted one-hot E for the whole group: one DVE multiply
                e4g = e4p.tile([128, gsize, EW], BF16, tag="e4g",
                               padded_shape=[128, GT, EW])
                nc.vector.tensor_mul(
                    e4g[:], s4_sb[:, gstart:gstart + gsize, :],
                    e_buf[:, gstart:gstart + gsize].broadcast_to(
                        [128, gsize, EW]))

                # run the PREVIOUS group's pooling now
                for fn in pending_pool:
                    fn()
                pending_pool = [
                    (lambda g=g, gsize=gsize, gstart=gstart, xn=xn, e4g=e4g:
                     emit_pool(g, gsize, gstart, xn, e4g))]
                gstart += gsize

            for fn in pending_pool:
                fn()

            nc.sync.dma_start(e_out_d[:], e_buf[:])

    nc.compile()
    return nc


def _prep_core(x, batch, r0, r1, seg0, nt, EW):
    rows = r1 - r0
    pad_rows = nt * 128

    xb = np.zeros((pad_rows, D), dtype=f8e3)
    xb[:rows] = x[r0:r1].astype(f8e3)
    # (128, nt, D): partition p holds row t*128 + p
    x_nat = np.ascontiguousarray(xb.reshape(nt, 128, D).transpose(1, 0, 2))

    # shipped quads' x^T, packed contiguously in consumption order:
    # xT[p, c, si*512 + r] = x[quad*512 + r, c*128 + p]
    shipped = _shipped_quads(nt)
    xq = xb.reshape(nt // QUAD, QUAD * 128, D)[shipped]      # (ns, 512, 256)
    xT = np.ascontiguousarray(
        xq.transpose(2, 0, 1).reshape(2, 128, len(shipped) * 512)
        .transpose(1, 0, 2))

    seg_local = np.full(pad_rows, -1, dtype=np.int64)
    seg_local[:rows] = batch[r0:r1] - seg0
    ci = np.where(seg_local < 0, -1.0, seg_local % EW).astype(np.float32)
    colidx = np.ascontiguousarray(ci.reshape(nt, 128).T)     # (128, nt) f32
    s4 = (colidx[:, :, None] ==
          np.arange(EW, dtype=np.float32)[None, None, :]).astype(f8e3)

    sizes = _group_plan(nt)
    gsegs = []  # per group: local segment ids present
    gstart = 0
    for g, gsize in enumerate(sizes):
        segs = np.unique(seg_local[gstart * 128:(gstart + gsize) * 128])
        segs = segs[segs >= 0]
        assert segs.size <= EW, f"group {g} spans {segs.size} segments > EW"
        gsegs.append(segs)
        gstart += gsize

    return {"x_nat": x_nat, "xT": xT, "s4": s4}, gsegs


def _shared_inputs(W1, W2, EW):
    w1t = np.ascontiguousarray(
        W1.T.astype(bf16).reshape(2, H, H).transpose(1, 0, 2))
    w2c = np.ascontiguousarray(W2.reshape(H, 1).astype(bf16))
    ident = np.eye(128, dtype=f8e3)
    return {"W1T": w1t, "W2c": w2c, "ident": ident}


def _seg_starts(x, batch):
    s = np.searchsorted(batch, np.arange(0, NCORES * SEGS_PER_CORE + 1,
                                         SEGS_PER_CORE))
    s[0], s[-1] = 0, x.shape[0]
    return s


def build_in_maps(x, batch, nt, EW=None):
    if EW is None:
        EW = pick_ew(x, batch, nt)
    s = _seg_starts(x, batch)
    maps, gsegs = [], []
    for c in range(NCORES):
        m, gs = _prep_core(x, batch, int(s[c]), int(s[c + 1]),
                           c * SEGS_PER_CORE, nt, EW)
        maps.append(m)
        gsegs.append(gs)
    return maps, gsegs


def pick_ew(x, batch, nt):
    """One-hot width: max segments any group spans on any core, +2 margin,
    rounded up to a multiple of 4 (capped at EW_CAP)."""
    s = _seg_starts(x, batch)
    sizes = _group_plan(nt)
    worst = 1
    for c in range(NCORES):
        r0, r1 = int(s[c]), int(s[c + 1])
        seg_local = np.full(nt * 128, -1, dtype=np.int64)
        seg_local[:r1 - r0] = np.asarray(batch[r0:r1]) - SEGS_PER_CORE * c
        gstart = 0
        for gsize in sizes:
            segs = np.unique(seg_local[gstart * 128:(gstart + gsize) * 128])
            worst = max(worst, (segs >= 0).sum())
            gstart += gsize
    return min(EW_CAP, -(-(int(worst) + 2) // 4) * 4)


def pick_nt(x, batch):
    s = _seg_starts(x, batch)
    nt = int(max(-(-(int(s[c + 1] - s[c])) // 128) for c in range(NCORES)))
    return -(-nt // QUAD) * QUAD


def kernel(x, batch, W1, W2, B):
    x = np.asarray(x)
    batch = np.asarray(batch)
    W1 = np.asarray(W1)
    W2 = np.asarray(W2)
    B = int(B)
    assert B == NCORES * SEGS_PER_CORE

    nt = pick_nt(x, batch)
    ew = pick_ew(x, batch, nt)
    key = (nt, ew)
    if key not in _kernel_cache:
        _kernel_cache.clear()
        _kernel_cache[key] = _build_kernel(nt, ew)
    nc = _kernel_cache[key]

    shared = _shared_inputs(W1, W2, ew)
    in_maps, gsegs = build_in_maps(x, batch, nt, ew)
    for m in in_maps:
        m.update(shared)

    res = run_bass_kernel_spmd(nc, in_maps, core_ids=list(range(NCORES)))

    seg_starts = _seg_starts(x, batch)
    z = np.empty((B, D), dtype=np.float32)
    for c in range(NCORES):
        # host-side scatter: gptout[g, d_lo, chunk, ew] -> num[seg, d]
        gpt = res.results[c]["gptout"].astype(np.float64)
        num = np.zeros((SEGS_PER_CORE, D))
        for g, segs in enumerate(gsegs[c]):
            sel = gpt[g][:, :, segs % ew]          # (128, 2, nJ)
            num[segs] += sel.transpose(2, 1, 0).reshape(len(segs), D)
        num = num.astype(np.float32)
        e = res.results[c]["e_out"].T.reshape(-1)
        r0, r1 = int(seg_starts[c]), int(seg_starts[c + 1])
        seg_local = (batch[r0:r1] - c * SEGS_PER_CORE).astype(np.int64)
        e_rows = e[:r1 - r0].astype(np.float64)
        den = np.bincount(seg_local, weights=e_rows, minlength=SEGS_PER_CORE)
        den = np.where(den == 0.0, 1.0, den).astype(np.float32)
        z[c * SEGS_PER_CORE:(c + 1) * SEGS_PER_CORE] = num / den[:, None]
    return z
